# revision 37
# baseline (speedup 1.0000x reference)
"""Cross-attention (pooling) Bass/Tile kernel for Trainium2.

Computation per batch element b (reference semantics):
    q      = query @ Wq.T + bq              [Q, D]
    ip     = tanh(input @ Wi.T + bi)        [S, D]
    scores = ip @ q.T                       [S, Q]
    attn   = softmax(0.3 * scores, axis=S)  [S, Q]
    out    = attn.T @ input                 [Q, D]
    return (out, attn)

Sharding: pure data-parallel over B=8 across the 8 NeuronCores (one batch
element per core); weights replicated.

Kernel strategy (per core):
  - All matmuls in fp32r (full PE rate at free-dim >= 256, ~12-bit mantissa),
    fp32 PSUM accumulation. Inputs are rounded to fp32r either by SWDGE
    DMA-cast on load or by the PSUM-eviction copy that follows each on-chip
    transpose.
  - Contractions need the contracted index on SBUF partitions for both
    operands, so Wq/Wi/query/input are transposed on-chip via PE transposes
    (fp32r transpose mode, 1.5 cyc/row).
  - Scores are produced in natural [S, Q] layout (S on partitions):
    attn is then directly usable as the attn output and as the stationary
    operand of out = attn.T @ input - no attn transposes.
  - Softmax over S skips max-subtraction (|0.3*scores| < ~40, far from fp32
    overflow). Column sums accumulate via a ones-vector matmul into one PSUM
    bank across all S tiles.
  - Unnormalized E = exp(0.3*scores) streams through per-chunk DRAM staging
    tiles; the second pass computes out_unnorm = E.T @ input on the PE
    (normalization applied at the very end as a per-partition scale by
    1/colsum), so the PE never waits on the softmax-normalization chain.
    attn = E * broadcast(1/colsum) runs on the Vector engine in parallel.
  - Phase B and phase C share SBUF tile tags (chunk shape == super shape),
    so there is no pool-release barrier between the passes and the phase-C
    prefetches can start while phase B drains.
"""

from contextlib import ExitStack

import numpy as np

import concourse.bacc as bacc
import concourse.tile as tile
from concourse import mybir
from concourse.masks import make_identity
from concourse.bass_utils import run_bass_kernel_spmd

P = 128
SOFTMAX_SCALE = 0.3
F32 = mybir.dt.float32
F32R = mybir.dt.float32r


def build_nc(S=4096, Q=1024, D=1024, CH=512):
    """Build + compile the single-core kernel (SPMD across cores).

    CH: streaming chunk in input rows (also the phase-C super size).
    """
    assert S % CH == 0 and CH % P == 0 and D % P == 0 and Q % P == 0
    NE = D // P          # e-tiles (projection output dim)
    ND = D // P          # d-tiles (projection contraction dim)
    NQT = Q // P         # q-tiles
    NST = S // P         # s-tiles
    TPC = CH // P        # s-tiles per chunk
    NCH = S // CH        # chunks (phase B) == supers (phase C)
    QHS = min(512, Q)
    NQH = Q // QHS
    DHS = min(512, D)
    NDH = D // DHS

    nc = bacc.Bacc("TRN2", target_bir_lowering=False, debug=False)

    query = nc.dram_tensor("query_tensor", (Q, D), F32, kind="ExternalInput")
    inp = nc.dram_tensor("input_tensor", (S, D), F32, kind="ExternalInput")
    wq_d = nc.dram_tensor("Wq", (D, D), F32, kind="ExternalInput")
    bq_d = nc.dram_tensor("bq", (D,), F32, kind="ExternalInput")
    wi_d = nc.dram_tensor("Wi", (D, D), F32, kind="ExternalInput")
    bi_d = nc.dram_tensor("bi", (D,), F32, kind="ExternalInput")
    out_d = nc.dram_tensor("out", (Q, D), F32, kind="ExternalOutput")
    attn_d = nc.dram_tensor("attn", (S, Q), F32, kind="ExternalOutput")

    Ident = mybir.ActivationFunctionType.Identity
    Tanh = mybir.ActivationFunctionType.Tanh
    Exp = mybir.ActivationFunctionType.Exp

    with tile.TileContext(nc) as tc, ExitStack() as ctx:
        singles = ctx.enter_context(tc.tile_pool(name="singles", bufs=1))
        dramp = ctx.enter_context(tc.tile_pool(name="dramp", bufs=1, space="DRAM"))
        mm_ps = ctx.enter_context(tc.tile_pool(name="mm_ps", bufs=4, space="PSUM"))
        tr_ps = ctx.enter_context(tc.tile_pool(name="tr_ps", bufs=2, space="PSUM"))
        cs_ps = ctx.enter_context(tc.tile_pool(name="cs_ps", bufs=1, space="PSUM"))
        # in_t tag serves phase-B chunk loads AND phase-C inB loads
        pin = ctx.enter_context(tc.tile_pool(name="pin", bufs=1))

        ident = singles.tile([P, P], F32)
        make_identity(nc, ident[:])
        identr = singles.tile([P, P], F32R)
        nc.vector.tensor_copy(identr[:], ident[:])
        ones_tmp = singles.tile([P, 1], F32)
        nc.vector.memset(ones_tmp[:], 1.0)
        ones_tmpr = singles.tile([1, P], F32)
        nc.vector.memset(ones_tmpr[:], 1.0)
        ones_col = singles.tile([P, 1], F32R)
        nc.vector.tensor_copy(ones_col[:], ones_tmp[:])
        ones_row = singles.tile([1, P], F32R)
        nc.vector.tensor_copy(ones_row[:], ones_tmpr[:])
        bias_sb = singles.tile([P, 2 * NE], F32)
        bq_sb = bias_sb[:, 0:NE]
        bi_sb = bias_sb[:, NE:2 * NE]
        nc.sync.dma_start(out=bq_sb, in_=bq_d[:].rearrange("(t p) -> p t", p=P))
        nc.sync.dma_start(out=bi_sb, in_=bi_d[:].rearrange("(t p) -> p t", p=P))

        # per-chunk staging of unnormalized exp (fp32r)
        e_dram = [dramp.tile([CH, Q], F32R, name=f"ed{c}", tag=f"ed{c}")
                  for c in range(NCH)]
        # per-chunk staging of the fp32r-cast input rows (SBUF layout, so the
        # phase-C reload is a plain same-dtype copy on any ring)
        inr_dram = [dramp.tile([P, TPC, D], F32R, name=f"ir{c}", tag=f"ir{c}")
                    for c in range(NCH)]

        def transpose_via_pe(dst_tiles, nat_aps, row0, dt, idn):
            """dst_tiles[d][:, (row0+k)*P...] = T(nat_aps[k][:, d-block])."""
            gn = len(nat_aps)
            for d in range(ND):
                pt = tr_ps.tile([P, gn * P], dt, name="pt", tag="tr")
                for k in range(gn):
                    nc.tensor.transpose(
                        pt[:, k * P:(k + 1) * P],
                        nat_aps[k][:, d * P:(d + 1) * P],
                        idn[:],
                    )
                nc.vector.tensor_copy(
                    dst_tiles[d][:, row0 * P:(row0 + gn) * P], pt[:]
                )

        def load_transposed(dst_tiles, src_ap, n_row_tiles, pool, tag):
            # fp32 HWDGE loads + fp32 PE transposes; the eviction copy rounds
            # to fp32r. Keeps phase A off the (cast-only) SWDGE ring.
            g0 = 0
            while g0 < n_row_tiles:
                gn = min(2, n_row_tiles - g0)
                nats = []
                for k in range(gn):
                    nat = pool.tile([P, D], F32, name="nat", tag=tag, bufs=7)
                    eng = nc.sync if (g0 + k) % 2 == 0 else nc.scalar
                    eng.dma_start(
                        out=nat[:], in_=src_ap[(g0 + k) * P:(g0 + k + 1) * P, :]
                    )
                    nats.append(nat)
                transpose_via_pe(dst_tiles, [n[:] for n in nats], g0, F32, ident)
                g0 += gn

        # persists through phase B (released before phase C needs SBUF)
        pw = tc.alloc_tile_pool(name="pw", bufs=1)
        if True:
            qT = [pw.tile([P, Q], F32R, name=f"qT{e}", tag=f"qT{e}") for e in range(NE)]
            WiT = [pw.tile([P, D], F32R, name=f"WiT{d}", tag=f"WiT{d}") for d in range(ND)]

            # ---------------- Phase A ----------------
            pa = tc.alloc_tile_pool(name="pa", bufs=1, side="right")
            if True:
                WqT = [pa.tile([P, D], F32R, name=f"WqT{d}", tag=f"WqT{d}") for d in range(ND)]
                qryT = [pa.tile([P, Q], F32R, name=f"qryT{d}", tag=f"qryT{d}") for d in range(ND)]
                load_transposed(WqT, wq_d[:], NE, pa, "natA")

                # interleave query-group loads with the q-projection on the
                # loaded columns so the PE works while the next group streams
                GW = 4 * P  # query-group width in q columns
                for g in range(Q // GW):
                    g0 = g * 4
                    nats = []
                    for k in range(4):
                        nat = pa.tile([P, D], F32, name="nat", tag="natA", bufs=7)
                        eng = nc.sync if k % 2 == 0 else nc.scalar
                        eng.dma_start(
                            out=nat[:],
                            in_=query[(g0 + k) * P:(g0 + k + 1) * P, :],
                        )
                        nats.append(nat)
                    transpose_via_pe(qryT, [n[:] for n in nats[0:2]], g0, F32, ident)
                    transpose_via_pe(qryT, [n[:] for n in nats[2:4]], g0 + 2, F32, ident)
                    for e in range(NE):
                        psq = mm_ps.tile([P, GW], F32, name="psq", tag="mm")
                        for d in range(ND):
                            nc.tensor.matmul(
                                psq[:],
                                WqT[d][:, e * P:(e + 1) * P],
                                qryT[d][:, g * GW:(g + 1) * GW],
                                start=(d == 0),
                                stop=(d == ND - 1),
                            )
                        nc.scalar.activation(
                            out=qT[e][:, g * GW:(g + 1) * GW],
                            in_=psq[:],
                            func=Ident,
                            bias=bq_sb[:, e:e + 1],
                            scale=1.0,
                        )

                load_transposed(WiT, wi_d[:], NE, pa, "natA")
            pa.release()

            # ---------------- Phase B: stream s-chunks ----------------
            cs_tiles = [
                cs_ps.tile([1, QHS], F32, name=f"cs{h}", tag=f"cs{h}")
                for h in range(NQH)
            ]
            pb = tc.alloc_tile_pool(name="pstream", bufs=1, side="right")
            if True:
                for c in range(NCH):
                    in_t = pin.tile([P, TPC, D], F32R, name="in_t", tag="in_t", bufs=3)
                    nc.gpsimd.dma_start(
                        out=in_t[:],
                        in_=inp[c * CH:(c + 1) * CH, :].rearrange(
                            "(t p) d -> p t d", p=P
                        ),
                    )
                    nc.gpsimd.dma_start(out=inr_dram[c][:], in_=in_t[:])
                    inputT = []
                    for d in range(ND):
                        pt = tr_ps.tile([P, CH], F32R, name="ptb", tag="tr")
                        for t in range(TPC):
                            nc.tensor.transpose(
                                pt[:, t * P:(t + 1) * P],
                                in_t[:, t, d * P:(d + 1) * P],
                                identr[:],
                            )
                        itd = pb.tile([P, CH], F32R, name="itd", tag="inputT", bufs=ND)
                        nc.vector.tensor_copy(itd[:], pt[:])
                        inputT.append(itd)
                    del pt
                    ipT = []
                    for e in range(NE):
                        ps = mm_ps.tile([P, CH], F32, name="psip", tag="mm")
                        for d in range(ND):
                            nc.tensor.matmul(
                                ps[:],
                                WiT[d][:, e * P:(e + 1) * P],
                                inputT[d][:],
                                start=(d == 0),
                                stop=(d == ND - 1),
                            )
                        ipe = pb.tile([P, CH], F32R, name="ipe", tag="ipT", bufs=NE + 2)
                        nc.scalar.activation(
                            out=ipe[:],
                            in_=ps[:],
                            func=Tanh,
                            bias=bi_sb[:, e:e + 1],
                            scale=1.0,
                        )
                        ipT.append(ipe)
                    E_t = pb.tile([P, TPC, Q], F32R, name="E_t", tag="E", bufs=3)
                    for t in range(TPC):
                        pss = [
                            mm_ps.tile([P, QHS], F32, name="pssc", tag="mm")
                            for _ in range(NQH)
                        ]
                        for e in range(NE):
                            for h in range(NQH):
                                nc.tensor.matmul(
                                    pss[h][:],
                                    ipT[e][:, t * P:(t + 1) * P],
                                    qT[e][:, h * QHS:(h + 1) * QHS],
                                    start=(e == 0),
                                    stop=(e == NE - 1),
                                )
                        for h in range(NQH):
                            nc.scalar.activation(
                                out=E_t[:, t, h * QHS:(h + 1) * QHS],
                                in_=pss[h][:],
                                func=Exp,
                                scale=SOFTMAX_SCALE,
                            )
                        st = c * TPC + t
                        for h in range(NQH):
                            nc.tensor.matmul(
                                cs_tiles[h][:],
                                ones_col[:],
                                E_t[:, t, h * QHS:(h + 1) * QHS],
                                start=(st == 0),
                                stop=(st == NST - 1),
                            )
                    nc.sync.dma_start(
                        out=e_dram[c][:].rearrange("(t p) q -> p t q", p=P),
                        in_=E_t[:],
                    )

                pw.release()

                # ------------- Phase C: attn out + out = E.T @ input --------
                pc = tc.alloc_tile_pool(name="pc", bufs=1)
                if True:
                    r_bcast = pc.tile([P, Q], F32)
                    cs_rowr = pb.tile([1, Q], F32R, name="cs_rowr", tag="csr", bufs=1)
                    # colsum -> R = broadcast(1/colsum), off the PE critical path
                    for h in range(NQH):
                        half = QHS // 2
                        nc.vector.tensor_copy(
                            cs_rowr[:, h * QHS:h * QHS + half], cs_tiles[h][:, 0:half]
                        )
                        nc.scalar.copy(
                            cs_rowr[:, h * QHS + half:(h + 1) * QHS],
                            cs_tiles[h][:, half:QHS],
                        )
                    for h in range(NQH):
                        pr = mm_ps.tile([P, QHS], F32, name="psr", tag="mm")
                        nc.tensor.matmul(
                            pr[:],
                            ones_row[:],
                            cs_rowr[:, h * QHS:(h + 1) * QHS],
                            start=True,
                            stop=True,
                        )
                        nc.vector.reciprocal(r_bcast[:, h * QHS:(h + 1) * QHS], pr[:])

                    # rT[:, qt] = 1/colsum for q-tile qt (per-partition layout)
                    csT_ps = tr_ps.tile([P, NQT], F32, name="csT_ps", tag="tr")
                    for qt in range(NQT):
                        nc.tensor.transpose(
                            csT_ps[:, qt:qt + 1],
                            cs_rowr[0:1, qt * P:(qt + 1) * P].bitcast(F32),
                            ident[0:1, 0:1],
                        )
                    csT = pc.tile([P, NQT], F32)
                    nc.vector.tensor_copy(csT[:], csT_ps[:])
                    rT = pc.tile([P, NQT], F32)
                    nc.vector.reciprocal(rT[:], csT[:])
                    out_sb = []
                    for t in range(NQT):
                        osb = pc.tile([P, D], F32, name=f"osb{t}", tag=f"osb{t}")
                        nc.gpsimd.memset(osb[:], 0.0)
                        out_sb.append(osb)
                    def issue_loads(sup):
                        eA = pb.tile([P, TPC, Q], F32R, name="eA", tag="E", bufs=3)
                        e_src = e_dram[sup][:].rearrange("(t p) q -> p t q", p=P)
                        nc.scalar.dma_start(
                            out=eA[:, :, 0:Q // 2], in_=e_src[:, :, 0:Q // 2]
                        )
                        nc.sync.dma_start(
                            out=eA[:, :, Q // 2:Q], in_=e_src[:, :, Q // 2:Q]
                        )
                        inB = pin.tile([P, TPC, D], F32R, name="inB", tag="in_t", bufs=3)
                        nc.sync.dma_start(
                            out=inB[:, 0:TPC // 2, :],
                            in_=inr_dram[sup][:, 0:TPC // 2, :],
                        )
                        nc.gpsimd.dma_start(
                            out=inB[:, TPC // 2:TPC, :],
                            in_=inr_dram[sup][:, TPC // 2:TPC, :],
                        )
                        return eA, inB

                    nxt = issue_loads(0)
                    for sup in range(NCH):
                        s0 = sup * CH
                        eA, inB = nxt
                        if sup + 1 < NCH:
                            nxt = issue_loads(sup + 1)
                        # attn output path (VectorE, off the PE critical path)
                        attn_t = pc.tile([P, TPC, Q], F32, name="attn_t",
                                         tag="attn_t", bufs=2)
                        for t in range(TPC):
                            nc.vector.tensor_mul(
                                attn_t[:, t, :], eA[:, t, :], r_bcast[:]
                            )
                        nc.gpsimd.dma_start(
                            out=attn_d[s0:s0 + CH, :].rearrange(
                                "(t p) q -> p t q", p=P
                            ),
                            in_=attn_t[:],
                        )
                        # out_unnorm accumulation on PE
                        for qt in range(NQT):
                            pso = [
                                mm_ps.tile([P, DHS], F32, name="pso", tag="mm")
                                for _ in range(NDH)
                            ]
                            for t in range(TPC):
                                for h in range(NDH):
                                    nc.tensor.matmul(
                                        pso[h][:],
                                        eA[:, t, qt * P:(qt + 1) * P],
                                        inB[:, t, h * DHS:(h + 1) * DHS],
                                        start=(t == 0),
                                        stop=(t == TPC - 1),
                                    )
                            for h in range(NDH):
                                # out_sb[qt] += psum * (1/colsum)[qt] - the
                                # normalization rides the eviction for free
                                nc.vector.scalar_tensor_tensor(
                                    out=out_sb[qt][:, h * DHS:(h + 1) * DHS],
                                    in0=pso[h][:],
                                    scalar=rT[:, qt:qt + 1],
                                    in1=out_sb[qt][:, h * DHS:(h + 1) * DHS],
                                    op0=mybir.AluOpType.mult,
                                    op1=mybir.AluOpType.add,
                                )
                            if sup == NCH - 1:
                                nc.sync.dma_start(
                                    out=out_d[qt * P:(qt + 1) * P, :],
                                    in_=out_sb[qt][:],
                                )
                pc.release()
            pb.release()

    nc.compile()
    return nc


_NC_CACHE = {}


def _get_nc(S=4096, Q=1024, D=1024):
    key = (S, Q, D)
    if key not in _NC_CACHE:
        _NC_CACHE[key] = build_nc(S=S, Q=Q, D=D)
    return _NC_CACHE[key]


def kernel(query_tensor, input_tensor, Wq, bq, Wi, bi, _trace=False, _tmpdir=None):
    """Full-input / full-output entry point; shards batch over 8 cores."""
    qt = np.ascontiguousarray(np.asarray(query_tensor, dtype=np.float32))
    it = np.ascontiguousarray(np.asarray(input_tensor, dtype=np.float32))
    wq = np.ascontiguousarray(np.asarray(Wq, dtype=np.float32))
    bqa = np.ascontiguousarray(np.asarray(bq, dtype=np.float32))
    wi = np.ascontiguousarray(np.asarray(Wi, dtype=np.float32))
    bia = np.ascontiguousarray(np.asarray(bi, dtype=np.float32))

    B, Q, D = qt.shape
    S = it.shape[1]
    assert B == 8, f"expected B=8, got {B}"
    nc = _get_nc(S=S, Q=Q, D=D)

    in_maps = [
        {
            "query_tensor": qt[b],
            "input_tensor": it[b],
            "Wq": wq,
            "bq": bqa,
            "Wi": wi,
            "bi": bia,
        }
        for b in range(B)
    ]
    try:
        res = run_bass_kernel_spmd(
            nc, in_maps, core_ids=list(range(B)), trace=_trace, tmpdir=_tmpdir
        )
    except Exception:
        # one retry: transient device wedges (NRT_EXEC_UNIT_UNRECOVERABLE)
        # have been observed to clear on re-dispatch
        res = run_bass_kernel_spmd(
            nc, in_maps, core_ids=list(range(B)), trace=_trace, tmpdir=_tmpdir
        )
    out = np.stack([res.results[b]["out"] for b in range(B)])
    attn = np.stack([res.results[b]["attn"] for b in range(B)])
    if _trace:
        kernel._last_results = res
    return out, attn


# revision 38
# speedup vs baseline: 1.0002x; 1.0002x over previous
"""Cross-attention (pooling) Bass/Tile kernel for Trainium2.

Computation per batch element b (reference semantics):
    q      = query @ Wq.T + bq              [Q, D]
    ip     = tanh(input @ Wi.T + bi)        [S, D]
    scores = ip @ q.T                       [S, Q]
    attn   = softmax(0.3 * scores, axis=S)  [S, Q]
    out    = attn.T @ input                 [Q, D]
    return (out, attn)

Sharding: pure data-parallel over B=8 across the 8 NeuronCores (one batch
element per core); weights replicated.

Kernel strategy (per core):
  - All matmuls in fp32r (full PE rate at free-dim >= 256, ~12-bit mantissa),
    fp32 PSUM accumulation. Inputs are rounded to fp32r either by SWDGE
    DMA-cast on load or by the PSUM-eviction copy that follows each on-chip
    transpose.
  - Contractions need the contracted index on SBUF partitions for both
    operands, so Wq/Wi/query/input are transposed on-chip via PE transposes
    (fp32r transpose mode, 1.5 cyc/row).
  - Scores are produced in natural [S, Q] layout (S on partitions):
    attn is then directly usable as the attn output and as the stationary
    operand of out = attn.T @ input - no attn transposes.
  - Softmax over S skips max-subtraction (|0.3*scores| < ~40, far from fp32
    overflow). Column sums accumulate via a ones-vector matmul into one PSUM
    bank across all S tiles.
  - Unnormalized E = exp(0.3*scores) streams through per-chunk DRAM staging
    tiles; the second pass computes out_unnorm = E.T @ input on the PE
    (normalization applied at the very end as a per-partition scale by
    1/colsum), so the PE never waits on the softmax-normalization chain.
    attn = E * broadcast(1/colsum) runs on the Vector engine in parallel.
  - Phase B and phase C share SBUF tile tags (chunk shape == super shape),
    so there is no pool-release barrier between the passes and the phase-C
    prefetches can start while phase B drains.
"""

from contextlib import ExitStack

import numpy as np

import concourse.bacc as bacc
import concourse.tile as tile
from concourse import mybir
from concourse.masks import make_identity
from concourse.bass_utils import run_bass_kernel_spmd

P = 128
SOFTMAX_SCALE = 0.3
F32 = mybir.dt.float32
F32R = mybir.dt.float32r


def build_nc(S=4096, Q=1024, D=1024, CH=512):
    """Build + compile the single-core kernel (SPMD across cores).

    CH: streaming chunk in input rows (also the phase-C super size).
    """
    assert S % CH == 0 and CH % P == 0 and D % P == 0 and Q % P == 0
    NE = D // P          # e-tiles (projection output dim)
    ND = D // P          # d-tiles (projection contraction dim)
    NQT = Q // P         # q-tiles
    NST = S // P         # s-tiles
    TPC = CH // P        # s-tiles per chunk
    NCH = S // CH        # chunks (phase B) == supers (phase C)
    QHS = min(512, Q)
    NQH = Q // QHS
    DHS = min(512, D)
    NDH = D // DHS

    nc = bacc.Bacc("TRN2", target_bir_lowering=False, debug=False)

    query = nc.dram_tensor("query_tensor", (Q, D), F32, kind="ExternalInput")
    inp = nc.dram_tensor("input_tensor", (S, D), F32, kind="ExternalInput")
    wq_d = nc.dram_tensor("Wq", (D, D), F32, kind="ExternalInput")
    bq_d = nc.dram_tensor("bq", (D,), F32, kind="ExternalInput")
    wi_d = nc.dram_tensor("Wi", (D, D), F32, kind="ExternalInput")
    bi_d = nc.dram_tensor("bi", (D,), F32, kind="ExternalInput")
    out_d = nc.dram_tensor("out", (Q, D), F32, kind="ExternalOutput")
    attn_d = nc.dram_tensor("attn", (S, Q), F32, kind="ExternalOutput")

    Ident = mybir.ActivationFunctionType.Identity
    Tanh = mybir.ActivationFunctionType.Tanh
    Exp = mybir.ActivationFunctionType.Exp

    with tile.TileContext(nc) as tc, ExitStack() as ctx:
        singles = ctx.enter_context(tc.tile_pool(name="singles", bufs=1))
        dramp = ctx.enter_context(tc.tile_pool(name="dramp", bufs=1, space="DRAM"))
        mm_ps = ctx.enter_context(tc.tile_pool(name="mm_ps", bufs=4, space="PSUM"))
        tr_ps = ctx.enter_context(tc.tile_pool(name="tr_ps", bufs=2, space="PSUM"))
        cs_ps = ctx.enter_context(tc.tile_pool(name="cs_ps", bufs=1, space="PSUM"))
        # in_t tag serves phase-B chunk loads AND phase-C inB loads
        pin = ctx.enter_context(tc.tile_pool(name="pin", bufs=1))

        ident = singles.tile([P, P], F32)
        make_identity(nc, ident[:])
        identr = singles.tile([P, P], F32R)
        nc.vector.tensor_copy(identr[:], ident[:])
        ones_tmp = singles.tile([P, 1], F32)
        nc.vector.memset(ones_tmp[:], 1.0)
        ones_tmpr = singles.tile([1, P], F32)
        nc.vector.memset(ones_tmpr[:], 1.0)
        ones_col = singles.tile([P, 1], F32R)
        nc.vector.tensor_copy(ones_col[:], ones_tmp[:])
        ones_row = singles.tile([1, P], F32R)
        nc.vector.tensor_copy(ones_row[:], ones_tmpr[:])
        bias_sb = singles.tile([P, 2 * NE], F32)
        bq_sb = bias_sb[:, 0:NE]
        bi_sb = bias_sb[:, NE:2 * NE]
        nc.sync.dma_start(out=bq_sb, in_=bq_d[:].rearrange("(t p) -> p t", p=P))
        nc.sync.dma_start(out=bi_sb, in_=bi_d[:].rearrange("(t p) -> p t", p=P))

        # per-chunk staging of unnormalized exp (fp32r)
        e_dram = [dramp.tile([CH, Q], F32R, name=f"ed{c}", tag=f"ed{c}")
                  for c in range(NCH)]
        # per-chunk staging of the fp32r-cast input rows (SBUF layout, so the
        # phase-C reload is a plain same-dtype copy on any ring)
        inr_dram = [dramp.tile([P, TPC, D], F32R, name=f"ir{c}", tag=f"ir{c}")
                    for c in range(NCH)]

        def transpose_via_pe(dst_tiles, nat_aps, row0, dt, idn):
            """dst_tiles[d][:, (row0+k)*P...] = T(nat_aps[k][:, d-block])."""
            gn = len(nat_aps)
            for d in range(ND):
                pt = tr_ps.tile([P, gn * P], dt, name="pt", tag="tr")
                for k in range(gn):
                    nc.tensor.transpose(
                        pt[:, k * P:(k + 1) * P],
                        nat_aps[k][:, d * P:(d + 1) * P],
                        idn[:],
                    )
                nc.vector.tensor_copy(
                    dst_tiles[d][:, row0 * P:(row0 + gn) * P], pt[:]
                )

        def load_transposed(dst_tiles, src_ap, n_row_tiles, pool, tag):
            # fp32 HWDGE loads + fp32 PE transposes; the eviction copy rounds
            # to fp32r. Keeps phase A off the (cast-only) SWDGE ring.
            g0 = 0
            while g0 < n_row_tiles:
                gn = min(2, n_row_tiles - g0)
                nats = []
                for k in range(gn):
                    nat = pool.tile([P, D], F32, name="nat", tag=tag, bufs=7)
                    eng = nc.sync if (g0 + k) % 2 == 0 else nc.scalar
                    eng.dma_start(
                        out=nat[:], in_=src_ap[(g0 + k) * P:(g0 + k + 1) * P, :]
                    )
                    nats.append(nat)
                transpose_via_pe(dst_tiles, [n[:] for n in nats], g0, F32, ident)
                g0 += gn

        # persists through phase B (released before phase C needs SBUF)
        pw = tc.alloc_tile_pool(name="pw", bufs=1)
        if True:
            qT = [pw.tile([P, Q], F32R, name=f"qT{e}", tag=f"qT{e}") for e in range(NE)]
            WiT = [pw.tile([P, D], F32R, name=f"WiT{d}", tag=f"WiT{d}") for d in range(ND)]

            # ---------------- Phase A ----------------
            pa = tc.alloc_tile_pool(name="pa", bufs=1, side="right")
            if True:
                WqT = [pa.tile([P, D], F32R, name=f"WqT{d}", tag=f"WqT{d}") for d in range(ND)]
                qryT = [pa.tile([P, Q], F32R, name=f"qryT{d}", tag=f"qryT{d}") for d in range(ND)]
                load_transposed(WqT, wq_d[:], NE, pa, "natA")

                # interleave query-group loads with the q-projection on the
                # loaded columns so the PE works while the next group streams
                GW = 4 * P  # query-group width in q columns
                for g in range(Q // GW):
                    g0 = g * 4
                    nats = []
                    for k in range(4):
                        nat = pa.tile([P, D], F32, name="nat", tag="natA", bufs=7)
                        eng = nc.sync if k % 2 == 0 else nc.scalar
                        eng.dma_start(
                            out=nat[:],
                            in_=query[(g0 + k) * P:(g0 + k + 1) * P, :],
                        )
                        nats.append(nat)
                    transpose_via_pe(qryT, [n[:] for n in nats[0:2]], g0, F32, ident)
                    transpose_via_pe(qryT, [n[:] for n in nats[2:4]], g0 + 2, F32, ident)
                    for e in range(NE):
                        psq = mm_ps.tile([P, GW], F32, name="psq", tag="mm")
                        for d in range(ND):
                            nc.tensor.matmul(
                                psq[:],
                                WqT[d][:, e * P:(e + 1) * P],
                                qryT[d][:, g * GW:(g + 1) * GW],
                                start=(d == 0),
                                stop=(d == ND - 1),
                            )
                        nc.scalar.activation(
                            out=qT[e][:, g * GW:(g + 1) * GW],
                            in_=psq[:],
                            func=Ident,
                            bias=bq_sb[:, e:e + 1],
                            scale=1.0,
                        )

                load_transposed(WiT, wi_d[:], NE, pa, "natA")
            pa.release()

            # ---------------- Phase B: stream s-chunks ----------------
            cs_tiles = [
                cs_ps.tile([1, QHS], F32, name=f"cs{h}", tag=f"cs{h}")
                for h in range(NQH)
            ]
            pb = tc.alloc_tile_pool(name="pstream", bufs=1, side="right")
            if True:
                for c in range(NCH):
                    in_t = pin.tile([P, TPC, D], F32R, name="in_t", tag="in_t", bufs=3)
                    nc.gpsimd.dma_start(
                        out=in_t[:],
                        in_=inp[c * CH:(c + 1) * CH, :].rearrange(
                            "(t p) d -> p t d", p=P
                        ),
                    )
                    nc.gpsimd.dma_start(out=inr_dram[c][:], in_=in_t[:])
                    inputT = []
                    for d in range(ND):
                        pt = tr_ps.tile([P, CH], F32R, name="ptb", tag="tr")
                        for t in range(TPC):
                            nc.tensor.transpose(
                                pt[:, t * P:(t + 1) * P],
                                in_t[:, t, d * P:(d + 1) * P],
                                identr[:],
                            )
                        itd = pb.tile([P, CH], F32R, name="itd", tag="inputT", bufs=ND)
                        nc.vector.tensor_copy(itd[:], pt[:])
                        inputT.append(itd)
                    del pt
                    ipT = []
                    for e in range(NE):
                        ps = mm_ps.tile([P, CH], F32, name="psip", tag="mm")
                        for d in range(ND):
                            nc.tensor.matmul(
                                ps[:],
                                WiT[d][:, e * P:(e + 1) * P],
                                inputT[d][:],
                                start=(d == 0),
                                stop=(d == ND - 1),
                            )
                        ipe = pb.tile([P, CH], F32R, name="ipe", tag="ipT", bufs=NE + 2)
                        nc.scalar.activation(
                            out=ipe[:],
                            in_=ps[:],
                            func=Tanh,
                            bias=bi_sb[:, e:e + 1],
                            scale=1.0,
                        )
                        ipT.append(ipe)
                    E_t = pb.tile([P, TPC, Q], F32R, name="E_t", tag="E", bufs=3)
                    for t in range(TPC):
                        pss = [
                            mm_ps.tile([P, QHS], F32, name="pssc", tag="mm")
                            for _ in range(NQH)
                        ]
                        for e in range(NE):
                            for h in range(NQH):
                                nc.tensor.matmul(
                                    pss[h][:],
                                    ipT[e][:, t * P:(t + 1) * P],
                                    qT[e][:, h * QHS:(h + 1) * QHS],
                                    start=(e == 0),
                                    stop=(e == NE - 1),
                                )
                        for h in range(NQH):
                            nc.scalar.activation(
                                out=E_t[:, t, h * QHS:(h + 1) * QHS],
                                in_=pss[h][:],
                                func=Exp,
                                scale=SOFTMAX_SCALE,
                            )
                        st = c * TPC + t
                        for h in range(NQH):
                            nc.tensor.matmul(
                                cs_tiles[h][:],
                                ones_col[:],
                                E_t[:, t, h * QHS:(h + 1) * QHS],
                                start=(st == 0),
                                stop=(st == NST - 1),
                            )
                    nc.sync.dma_start(
                        out=e_dram[c][:].rearrange("(t p) q -> p t q", p=P),
                        in_=E_t[:],
                    )

                pw.release()

                # ------------- Phase C: attn out + out = E.T @ input --------
                pc = tc.alloc_tile_pool(name="pc", bufs=1)
                if True:
                    r_bcast = pc.tile([P, Q], F32)
                    cs_rowr = pb.tile([1, Q], F32R, name="cs_rowr", tag="csr", bufs=1)
                    # colsum -> R = broadcast(1/colsum), off the PE critical path
                    for h in range(NQH):
                        half = QHS // 2
                        nc.vector.tensor_copy(
                            cs_rowr[:, h * QHS:h * QHS + half], cs_tiles[h][:, 0:half]
                        )
                        nc.scalar.copy(
                            cs_rowr[:, h * QHS + half:(h + 1) * QHS],
                            cs_tiles[h][:, half:QHS],
                        )
                    for h in range(NQH):
                        pr = mm_ps.tile([P, QHS], F32, name="psr", tag="mm")
                        nc.tensor.matmul(
                            pr[:],
                            ones_row[:],
                            cs_rowr[:, h * QHS:(h + 1) * QHS],
                            start=True,
                            stop=True,
                        )
                        nc.vector.reciprocal(r_bcast[:, h * QHS:(h + 1) * QHS], pr[:])

                    # rT[:, qt] = 1/colsum for q-tile qt (per-partition layout)
                    csT_ps = tr_ps.tile([P, NQT], F32, name="csT_ps", tag="tr")
                    for qt in range(NQT):
                        nc.tensor.transpose(
                            csT_ps[:, qt:qt + 1],
                            cs_rowr[0:1, qt * P:(qt + 1) * P].bitcast(F32),
                            ident[0:1, 0:1],
                        )
                    csT = pc.tile([P, NQT], F32)
                    nc.vector.tensor_copy(csT[:], csT_ps[:])
                    rT = pc.tile([P, NQT], F32)
                    nc.vector.reciprocal(rT[:], csT[:])
                    out_sb = []
                    for t in range(NQT):
                        osb = pc.tile([P, D], F32, name=f"osb{t}", tag=f"osb{t}")
                        nc.gpsimd.memset(osb[:], 0.0)
                        out_sb.append(osb)
                    def issue_loads(sup):
                        eA = pb.tile([P, TPC, Q], F32R, name="eA", tag="E", bufs=3)
                        e_src = e_dram[sup][:].rearrange("(t p) q -> p t q", p=P)
                        nc.scalar.dma_start(
                            out=eA[:, :, 0:Q // 2], in_=e_src[:, :, 0:Q // 2]
                        )
                        nc.sync.dma_start(
                            out=eA[:, :, Q // 2:Q], in_=e_src[:, :, Q // 2:Q]
                        )
                        inB = pin.tile([P, TPC, D], F32R, name="inB", tag="in_t", bufs=3)
                        nc.sync.dma_start(
                            out=inB[:, 0:TPC // 2, :],
                            in_=inr_dram[sup][:, 0:TPC // 2, :],
                        )
                        nc.gpsimd.dma_start(
                            out=inB[:, TPC // 2:TPC, :],
                            in_=inr_dram[sup][:, TPC // 2:TPC, :],
                        )
                        return eA, inB

                    nxt = issue_loads(0)
                    for sup in range(NCH):
                        s0 = sup * CH
                        eA, inB = nxt
                        if sup + 1 < NCH:
                            nxt = issue_loads(sup + 1)
                        # attn output path (VectorE, off the PE critical path)
                        attn_t = pc.tile([P, TPC, Q], F32, name="attn_t",
                                         tag="attn_t", bufs=2)
                        for t in range(TPC):
                            nc.vector.tensor_mul(
                                attn_t[:, t, :], eA[:, t, :], r_bcast[:]
                            )
                        nc.gpsimd.dma_start(
                            out=attn_d[s0:s0 + CH, :].rearrange(
                                "(t p) q -> p t q", p=P
                            ),
                            in_=attn_t[:],
                        )
                        # out_unnorm accumulation on PE
                        for qt in range(NQT):
                            pso = [
                                mm_ps.tile([P, DHS], F32, name="pso", tag="mm")
                                for _ in range(NDH)
                            ]
                            for t in range(TPC):
                                for h in range(NDH):
                                    nc.tensor.matmul(
                                        pso[h][:],
                                        eA[:, t, qt * P:(qt + 1) * P],
                                        inB[:, t, h * DHS:(h + 1) * DHS],
                                        start=(t == 0),
                                        stop=(t == TPC - 1),
                                    )
                            for h in range(NDH):
                                # out_sb[qt] += psum * (1/colsum)[qt] - the
                                # normalization rides the eviction for free
                                nc.vector.scalar_tensor_tensor(
                                    out=out_sb[qt][:, h * DHS:(h + 1) * DHS],
                                    in0=pso[h][:],
                                    scalar=rT[:, qt:qt + 1],
                                    in1=out_sb[qt][:, h * DHS:(h + 1) * DHS],
                                    op0=mybir.AluOpType.mult,
                                    op1=mybir.AluOpType.add,
                                )
                            if sup == NCH - 1:
                                # spread the tail stores across all three
                                # DMA rings so the drain isn't ring-serial
                                oeng = (nc.sync, nc.scalar, nc.gpsimd)[qt % 3]
                                oeng.dma_start(
                                    out=out_d[qt * P:(qt + 1) * P, :],
                                    in_=out_sb[qt][:],
                                )
                pc.release()
            pb.release()

    nc.compile()
    return nc


_NC_CACHE = {}


def _get_nc(S=4096, Q=1024, D=1024):
    key = (S, Q, D)
    if key not in _NC_CACHE:
        _NC_CACHE[key] = build_nc(S=S, Q=Q, D=D)
    return _NC_CACHE[key]


def kernel(query_tensor, input_tensor, Wq, bq, Wi, bi, _trace=False, _tmpdir=None):
    """Full-input / full-output entry point; shards batch over 8 cores."""
    qt = np.ascontiguousarray(np.asarray(query_tensor, dtype=np.float32))
    it = np.ascontiguousarray(np.asarray(input_tensor, dtype=np.float32))
    wq = np.ascontiguousarray(np.asarray(Wq, dtype=np.float32))
    bqa = np.ascontiguousarray(np.asarray(bq, dtype=np.float32))
    wi = np.ascontiguousarray(np.asarray(Wi, dtype=np.float32))
    bia = np.ascontiguousarray(np.asarray(bi, dtype=np.float32))

    B, Q, D = qt.shape
    S = it.shape[1]
    assert B == 8, f"expected B=8, got {B}"
    nc = _get_nc(S=S, Q=Q, D=D)

    in_maps = [
        {
            "query_tensor": qt[b],
            "input_tensor": it[b],
            "Wq": wq,
            "bq": bqa,
            "Wi": wi,
            "bi": bia,
        }
        for b in range(B)
    ]
    try:
        res = run_bass_kernel_spmd(
            nc, in_maps, core_ids=list(range(B)), trace=_trace, tmpdir=_tmpdir
        )
    except Exception:
        # one retry: transient device wedges (NRT_EXEC_UNIT_UNRECOVERABLE)
        # have been observed to clear on re-dispatch
        res = run_bass_kernel_spmd(
            nc, in_maps, core_ids=list(range(B)), trace=_trace, tmpdir=_tmpdir
        )
    out = np.stack([res.results[b]["out"] for b in range(B)])
    attn = np.stack([res.results[b]["attn"] for b in range(B)])
    if _trace:
        kernel._last_results = res
    return out, attn
